# revision 22
# baseline (speedup 1.0000x reference)
"""Trainium2 Bass kernel for LocalSpatialSimilarity.

Per sample (B=16, C=256, H=W=64, N=4096 pixels):
  s[p]  = sum_c x[c,p]                (channel sum)
  q[p]  = sum_c x[c,p]^2              (channel sum of squares)
  box   = 3x3 zero-padded box-sum of s (reshaped to 64x64)
  D     = max(box^2 * q * 256/81, 1e-12)
  sim   = (box * s / 9) * rsqrt(D)
  out   = softmax over p of (mask ? -inf : -sim)
        = (mask ? 0 : exp(-sim)) / total        (sim bounded in [-1,1] -> no
                                                 max-subtraction needed)

Sharding: pure data parallel, 2 samples per core across 8 cores.

Pipeline: x streams in as eleven [128, <=2048] float32r pieces, all on the
sync HWDGE ring with nothing ahead of them (a single clean ring measures
~350 GB/s; splitting across both rings thrashed HBM down to ~210).  Per
piece: channel-sum matmuls on the tensor engine straight off the raw piece
(float32r, one PE pass), squares on scalar/vector engines, then
sum-of-squares matmuls.  The band stationary trick lands 512-pixel blocks on
psum partitions so each sample accumulates into one [8, 512] psum tile per
quantity.  The last pieces shrink to 256 pixels so the post-stream tail is
short.

The spatial phase runs per sample.  Sample 0's phase hides under sample 1's
DMA stream; its small DMAs ride SWDGE (gpsimd) so the compute rings stay
clean, and its exp/normalize tail is emitted in a mid-stream slot so the
in-order scalar/vector engines never stall on it.  rsqrt is a Quake-style
int-shift seed plus one scalar_tensor_tensor-fused Newton step on the vector
engine, which keeps every scalar-engine function inside the exp_and_others
activation-table set: exactly one table load, none on the critical path.
"""

import sys

sys.path.insert(0, "/opt/trn_rl_repo")

import numpy as np

import concourse.bacc as bacc
import concourse.mybir as mybir
import concourse.tile as tile
from concourse.bass_utils import run_bass_kernel_spmd

B, C, H, W = 16, 256, 64, 64
N = H * W
NCORES = 8
SPC = B // NCORES  # samples per core
EPS2 = 1e-12
FP32 = mybir.dt.float32
I32 = mybir.dt.int32

# float32r: relaxed-precision fp32 matmul, single PE pass (plain fp32 = two).
MM_DT = mybir.dt.float32r
# Quake rsqrt seed: y = bitcast((0xBE6EB3BE - bitcast(D)) >> 1), one NR step.
QUAKE_2MAGIC = 0xBE6EB3BE  # 2 * 0x5F3759DF
NR_STEPS = 1  # one Newton step: 4.4e-4 output err (seed-only: 8.5e-3)

AF = mybir.ActivationFunctionType
ALU = mybir.AluOpType

# x pieces (sample, channel-chunk, pixel offset, length) in stream order.
# Sample SPC-1's chunk-1 pieces shrink toward the end: the last piece gates
# the kernel tail.
_PIECES = []
for _s in range(SPC):
    for _c in range(2):
        if _s == SPC - 1 and _c == 1:
            spans = [(0, 2048), (2048, 1024), (3072, 512), (3584, 256), (3840, 256)]
        else:
            spans = [(0, 2048), (2048, 2048)]
        for _o, _l in spans:
            _PIECES.append((_s, _c, _o, _l))
_LAST = len(_PIECES) - 1

# All x pieces ride the sync ring: splitting the stream across both HWDGE
# rings was measured MUCH slower (two packet-interleaved streams thrash HBM:
# 211 GB/s combined vs 294 single-ring).  The scalar ring stays empty for
# the tail sample's low-latency reshape/store smalls.
_SYNC_KS = set(range(len(_PIECES)))
# Square engine: vector for sample-0's odd pieces and the last piece,
# scalar for the rest (the vector engine runs the spatial chains, so
# sample-1 squares must not queue behind them in program order).
_DVE_SQ_KS = {1, 3, _LAST}


def _kernel_body(ctx, tc, x, mask32, vband, hband, out):
    nc = tc.nc

    consts = ctx.enter_context(tc.tile_pool(name="consts", bufs=1))
    xp = ctx.enter_context(tc.tile_pool(name="xp", bufs=len(_PIECES)))
    sqp = ctx.enter_context(tc.tile_pool(name="sqp", bufs=3))
    rows = ctx.enter_context(tc.tile_pool(name="rows", bufs=4))
    sm = ctx.enter_context(tc.tile_pool(name="sm", bufs=2))
    psa = ctx.enter_context(tc.tile_pool(name="psa", bufs=2 * SPC, space="PSUM"))
    pss = ctx.enter_context(tc.tile_pool(name="pss", bufs=2, space="PSUM"))

    # ---- phase 0: input streams + constants ------------------------------
    xts = []
    for k, (s, c, o, ln) in enumerate(_PIECES):
        xt = xp.tile([128, ln], MM_DT, tag="x", name=f"x{k}")
        eng = nc.sync if k in _SYNC_KS else nc.scalar
        eng.dma_start(out=xt[:], in_=x[s, 128 * c : 128 * (c + 1), o : o + ln])
        xts.append(xt)

    # Constants ride SWDGE so the two HWDGE rings stay pure x-stream.
    band = consts.tile([128, 15], MM_DT)
    nc.gpsimd.dma_start(out=band[:], in_=hband.ap())
    band64 = consts.tile([64, 64], FP32)
    nc.gpsimd.dma_start(out=band64[:], in_=vband.ap())
    maskt = consts.tile([64, SPC, 64], FP32)
    nc.gpsimd.dma_start(out=maskt[:], in_=mask32.ap())

    ones64 = consts.tile([64, 64], FP32)
    nc.gpsimd.memset(ones64[:], 1.0)

    # Column-padded S tiles (cols 0 and 65 stay zero) and the shared rowsum.
    sbp = [consts.tile([64, 66], FP32, tag=f"sbp{s}", name=f"sbp{s}") for s in range(SPC)]
    for s in range(SPC):
        nc.gpsimd.memset(sbp[s][:, 0:1], 0.0)
        nc.gpsimd.memset(sbp[s][:, 65:66], 0.0)
    rowsum = consts.tile([64, SPC], FP32)

    ps_s = [psa.tile([8, 512], FP32, tag="acc", name=f"ps_s{i}") for i in range(SPC)]
    ps_q = [psa.tile([8, 512], FP32, tag="acc", name=f"ps_q{i}") for i in range(SPC)]

    spat = {}  # per-sample state carried from part 1 to part 2

    # ---- phase A: streamed channel reductions ----------------------------
    done = [0] * SPC  # per-sample pixels*chunks streamed so far (full = 2N)
    for k, (s, c, o, ln) in enumerate(_PIECES):
        xt = xts[k]
        first, last = done[s] == 0, done[s] + ln == 2 * N
        nblk = (ln + 511) // 512
        for b in range(nblk):
            f0, f1 = 512 * b, min(512 * b + 512, ln)
            j = (o + f0) // 512
            lo = o + f0 - 512 * j
            # s-sums in exact fp32 (two PE passes): box-sums cancel near
            # zero at some pixels and fp32r noise there flips sign(box),
            # which flips sim.  q stays fp32r (all-positive, benign).
            nc.tensor.matmul(
                ps_s[s][:, lo : lo + f1 - f0],
                band[:, 7 - j : 15 - j].bitcast(FP32),
                xt[:, f0:f1].bitcast(FP32),
                start=first and b == 0,
                stop=last and b == nblk - 1,
            )
        sq = sqp.tile([128, ln], MM_DT, tag="sq", name=f"sq{k}")
        xf = xt[:].bitcast(FP32)
        if k in _DVE_SQ_KS:
            nc.vector.tensor_mul(sq[:], xf, xf)
        else:
            nc.scalar.activation(sq[:], xf, AF.Square)
        for b in range(nblk):
            f0, f1 = 512 * b, min(512 * b + 512, ln)
            j = (o + f0) // 512
            lo = o + f0 - 512 * j
            nc.tensor.matmul(
                ps_q[s][:, lo : lo + f1 - f0],
                band[:, 7 - j : 15 - j],
                sq[:, f0:f1],
                start=first and b == 0,
                stop=last and b == nblk - 1,
            )
        done[s] += ln
        if done[s] == 2 * N:
            _spatial_a(tc, spat, s, ps_s[s], ps_q[s], band64, sbp[s], rows, sm, pss)
            _spatial_b(tc, spat, s, maskt, sm)
            if s == SPC - 1:
                _spatial_finish(tc, spat, s, ones64, rowsum, sm, pss, out)
        elif s == SPC - 1 and done[s] == N:
            # Mid-stream slot: earlier samples' exp/normalize/store, placed
            # here so the scalar stream reaches it after the dependencies
            # (their vector chains) have cleared.
            for t in range(SPC - 1):
                _spatial_finish(tc, spat, t, ones64, rowsum, sm, pss, out)


def _spatial_a(tc, spat, s, ps_s, ps_q, band64, sbp, rows, sm, pss):
    """Reshape + box filter for one sample, through BOX and T = box*s.

    The horizontal taps and cosine chain live in _spatial_b."""
    nc = tc.nc
    tail = s == SPC - 1

    # PSUM -> SBUF, then reshape [8,512] -> [64,64] (both APs enumerate the
    # 4096 pixels in order).  The s-reshape lands in the padded S tile.
    s_sb = rows.tile([8, 512], FP32, tag="srow", name=f"s_sb{s}")
    nc.vector.tensor_copy(s_sb[:], ps_s[:])
    q_sb = rows.tile([8, 512], FP32, tag="qrow", name=f"q_sb{s}")
    nc.scalar.copy(q_sb[:], ps_q[:])
    Qt = sm.tile([64, 64], FP32, tag="Qt", name=f"Qt{s}")
    if tail:
        nc.sync.dma_start(out=sbp[:, 1:65], in_=s_sb[:])
        nc.scalar.dma_start(out=Qt[:], in_=q_sb[:])
    else:
        nc.gpsimd.dma_start(out=sbp[:, 1:65], in_=s_sb[:])
        nc.gpsimd.dma_start(out=Qt[:], in_=q_sb[:])

    # Vertical 3-tap via tridiagonal stationary (pad columns stay zero
    # through the matmul), horizontal taps via free-shifted adds.
    v_ps = pss.tile([64, 66], FP32, tag="spat", name=f"v_ps{s}")
    nc.tensor.matmul(v_ps[:], band64[:], sbp[:], start=True, stop=True)
    Hb = sm.tile([64, 66], FP32, tag="Hb", name=f"Hb{s}")
    nc.scalar.copy(Hb[:], v_ps[:])
    spat[s] = (Hb, Qt, sbp)


def _spatial_b(tc, spat, s, maskt, sm):
    """Cosine chain for one sample: D, quake rsqrt, through exp input U2.

    Always on the vector engine; for non-tail samples this is emitted in a
    mid-stream slot so squares never queue behind it."""
    nc = tc.nc
    Hb, Qt, sbp = spat[s]

    # Horizontal 3-tap via free-shifted adds of the vertical-sum tile.
    T1 = sm.tile([64, 64], FP32, tag="T1", name=f"T1{s}")
    nc.vector.tensor_add(T1[:], Hb[:, 0:64], Hb[:, 1:65])
    BOX = sm.tile([64, 64], FP32, tag="BOX", name=f"BOX{s}")
    nc.vector.tensor_add(BOX[:], T1[:], Hb[:, 2:66])
    # T = box * s off the critical chain on gpsimd.
    T = sm.tile([64, 64], FP32, tag="T", name=f"T{s}")
    nc.gpsimd.tensor_mul(T[:], BOX[:], sbp[:, 1:65])

    # D = max(box^2 * q * 256/81, eps^2)
    P = sm.tile([64, 64], FP32, tag="P", name=f"P{s}")
    nc.vector.scalar_tensor_tensor(
        P[:], BOX[:], 256.0 / 81.0, BOX[:], op0=ALU.mult, op1=ALU.mult
    )
    P2 = sm.tile([64, 64], FP32, tag="P2", name=f"P2{s}")
    nc.vector.tensor_mul(P2[:], P[:], Qt[:])
    Dt = sm.tile([64, 64], FP32, tag="Dt", name=f"Dt{s}")
    nc.vector.tensor_scalar_max(Dt[:], P2[:], EPS2)

    # rsqrt(D): quake int-shift seed + one fused Newton step.
    ti = sm.tile([64, 64], I32, tag="ti", name=f"ti{s}")
    nc.vector.tensor_scalar(
        ti[:], Dt[:].bitcast(I32), 1, None, op0=ALU.logical_shift_right
    )
    yi = sm.tile([64, 64], I32, tag="yi", name=f"yi{s}")
    nc.vector.tensor_scalar(
        yi[:], ti[:], -1, QUAKE_2MAGIC // 2, op0=ALU.mult, op1=ALU.add
    )
    y0 = yi[:].bitcast(FP32)
    if NR_STEPS:
        a = sm.tile([64, 64], FP32, tag="nra", name=f"nra{s}")
        nc.vector.tensor_mul(a[:], y0, y0)
        w = sm.tile([64, 64], FP32, tag="nrw", name=f"nrw{s}")
        nc.vector.scalar_tensor_tensor(
            w[:], a[:], -0.5, Dt[:], op0=ALU.mult, op1=ALU.mult
        )
        y1 = sm.tile([64, 64], FP32, tag="nry", name=f"nry{s}")
        nc.vector.scalar_tensor_tensor(
            y1[:], w[:], 1.5, y0, op0=ALU.add, op1=ALU.mult
        )
        y0 = y1[:]

    # U2 = box*s*rsqrt + 1e30*mask  (exp(-U2/9) = masked exp(-sim))
    U = sm.tile([64, 64], FP32, tag="U", name=f"U{s}")
    nc.vector.tensor_mul(U[:], T[:], y0)
    U2 = sm.tile([64, 64], FP32, tag="U2", name=f"U2{s}")
    nc.vector.tensor_add(U2[:], U[:], maskt[:, s, :])
    spat[s] = U2


def _spatial_finish(tc, spat, s, ones64, rowsum, sm, pss, out):
    """exp, per-sample total, normalize, store."""
    nc = tc.nc
    U2 = spat[s]
    EM = sm.tile([64, 64], FP32, tag="EM", name=f"EM{s}")
    nc.scalar.activation(
        EM[:], U2[:], AF.Exp, scale=-1.0 / 9.0, accum_out=rowsum[:, s : s + 1]
    )
    # Total broadcast to all 64 partitions in one ones-matmul.
    totb = pss.tile([64, 1], FP32, tag="spat", name=f"totb{s}")
    nc.tensor.matmul(totb[:], ones64[:], rowsum[:, s : s + 1], start=True, stop=True)
    rec = sm.tile([64, 1], FP32, tag="rec", name=f"rec{s}")
    nc.vector.reciprocal(rec[:], totb[:])
    OUTt = sm.tile([64, 64], FP32, tag="OUTt", name=f"OUTt{s}")
    nc.vector.tensor_scalar_mul(OUTt[:], EM[:], rec[:, 0:1])
    eng = nc.scalar if s == SPC - 1 else nc.gpsimd
    eng.dma_start(out=out[:, s, :], in_=OUTt[:])


_NC_CACHE = {}


def _build():
    key = f"v4-nr{NR_STEPS}"
    if key in _NC_CACHE:
        return _NC_CACHE[key]
    nc = bacc.Bacc("TRN2", target_bir_lowering=False, debug=False)
    x = nc.declare_dram_parameter("x", [SPC, C, N], MM_DT, isOutput=False)
    mask32 = nc.declare_dram_parameter("mask32", [64, SPC, 64], FP32, isOutput=False)
    vband = nc.declare_dram_parameter("vband", [64, 64], FP32, isOutput=False)
    hband = nc.declare_dram_parameter("hband", [128, 15], MM_DT, isOutput=False)
    out = nc.declare_dram_parameter("out", [64, SPC, 64], FP32, isOutput=True)
    from contextlib import ExitStack

    with tile.TileContext(nc) as tc, ExitStack() as ctx:
        _kernel_body(ctx, tc, x, mask32, vband, hband, out)
    nc.compile()
    _NC_CACHE[key] = nc
    return nc


def band_matrix() -> np.ndarray:
    idx = np.arange(64)
    return (np.abs(idx[:, None] - idx[None, :]) <= 1).astype(np.float32)


def ind_band() -> np.ndarray:
    b = np.zeros((128, 15), dtype=np.float32)
    b[:, 7] = 1.0
    return b


def make_in_maps(x: np.ndarray, prev_drop_mask: np.ndarray) -> list:
    xs = np.ascontiguousarray(np.asarray(x), dtype=np.float32).reshape(B, C, N)
    # [B, N] bool -> per-core [64 rows, SPC, 64 cols] f32 pre-scaled +1e30.
    m32 = (np.asarray(prev_drop_mask).astype(np.float32) * 1e30).reshape(B, H, W)
    vb = band_matrix()
    hb = ind_band()
    maps = []
    for i in range(NCORES):
        mc = np.ascontiguousarray(m32[i * SPC : (i + 1) * SPC].transpose(1, 0, 2))
        maps.append(
            {
                "x": xs[i * SPC : (i + 1) * SPC],
                "mask32": mc,
                "vband": vb,
                "hband": hb,
            }
        )
    return maps


def gather_out(results) -> np.ndarray:
    # Each core returns [64, SPC, 64]; reorder to [B, H, W].
    outs = [
        np.asarray(results[i]["out"]).transpose(1, 0, 2) for i in range(NCORES)
    ]
    return np.concatenate(outs, axis=0).reshape(B, H, W)


def kernel(x: np.ndarray, prev_drop_mask: np.ndarray) -> np.ndarray:
    nc = _build()
    res = run_bass_kernel_spmd(nc, make_in_maps(x, prev_drop_mask), list(range(NCORES)))
    return gather_out(res.results)


# revision 23
# speedup vs baseline: 1.2215x; 1.2215x over previous
"""Trainium2 Bass kernel for LocalSpatialSimilarity.

Per sample (B=16, C=256, H=W=64, N=4096 pixels):
  s[p]  = sum_c x[c,p]                (channel sum)
  q[p]  = sum_c x[c,p]^2              (channel sum of squares)
  box   = 3x3 zero-padded box-sum of s (reshaped to 64x64)
  D     = max(box^2 * q * 256/81, 1e-12)
  sim   = (box * s / 9) * rsqrt(D)
  out   = softmax over p of (mask ? -inf : -sim)
        = (mask ? 0 : exp(-sim)) / total        (sim bounded in [-1,1] -> no
                                                 max-subtraction needed)

Sharding: pure data parallel, 2 samples per core across 8 cores.

Pipeline: x streams in as eleven [128, <=2048] float32r pieces, all on the
sync HWDGE ring with nothing ahead of them (a single clean ring measures
~350 GB/s; splitting across both rings thrashed HBM down to ~210).  Per
piece: channel-sum matmuls on the tensor engine straight off the raw piece
(float32r, one PE pass), squares on scalar/vector engines, then
sum-of-squares matmuls.  The band stationary trick lands 512-pixel blocks on
psum partitions so each sample accumulates into one [8, 512] psum tile per
quantity.  The last pieces shrink to 256 pixels so the post-stream tail is
short.

The spatial phase runs per sample.  Sample 0's phase hides under sample 1's
DMA stream; its small DMAs ride SWDGE (gpsimd) so the compute rings stay
clean, and its exp/normalize tail is emitted in a mid-stream slot so the
in-order scalar/vector engines never stall on it.  rsqrt is a Quake-style
int-shift seed plus one scalar_tensor_tensor-fused Newton step on the vector
engine, which keeps every scalar-engine function inside the exp_and_others
activation-table set: exactly one table load, none on the critical path.
"""

import sys

sys.path.insert(0, "/opt/trn_rl_repo")

import numpy as np

import concourse.bacc as bacc
import concourse.mybir as mybir
import concourse.tile as tile
from concourse.bass_utils import run_bass_kernel_spmd

B, C, H, W = 16, 256, 64, 64
N = H * W
NCORES = 8
SPC = B // NCORES  # samples per core
EPS2 = 1e-12
FP32 = mybir.dt.float32
I32 = mybir.dt.int32

# float32r: relaxed-precision fp32 matmul, single PE pass (plain fp32 = two).
MM_DT = mybir.dt.float32r
# Quake rsqrt seed: y = bitcast((0xBE6EB3BE - bitcast(D)) >> 1), one NR step.
QUAKE_2MAGIC = 0xBE6EB3BE  # 2 * 0x5F3759DF
NR_STEPS = 1  # one Newton step: 4.4e-4 output err (seed-only: 8.5e-3)

AF = mybir.ActivationFunctionType
ALU = mybir.AluOpType

# x pieces (sample, channel-chunk, pixel offset, length) in stream order.
# Sample SPC-1's chunk-1 pieces shrink toward the end: the last piece gates
# the kernel tail.
_PIECES = []
for _s in range(SPC):
    for _c in range(2):
        if _s == SPC - 1 and _c == 1:
            spans = [(0, 2048), (2048, 1024), (3072, 512), (3584, 256), (3840, 256)]
        else:
            spans = [(0, 2048), (2048, 2048)]
        for _o, _l in spans:
            _PIECES.append((_s, _c, _o, _l))
_LAST = len(_PIECES) - 1

# All x pieces ride the sync ring: splitting the stream across both HWDGE
# rings was measured MUCH slower (two packet-interleaved streams thrash HBM:
# 211 GB/s combined vs 294 single-ring).  The scalar ring stays empty for
# the tail sample's low-latency reshape/store smalls.
_SYNC_KS = set(range(len(_PIECES)))
# Square engine: vector for sample-0's odd pieces and the last piece,
# scalar for the rest (the vector engine runs the spatial chains, so
# sample-1 squares must not queue behind them in program order).
_DVE_SQ_KS = {1, 3, _LAST}


def _kernel_body(ctx, tc, x, mask32, vband, hband, out):
    nc = tc.nc

    consts = ctx.enter_context(tc.tile_pool(name="consts", bufs=1))
    xp = ctx.enter_context(tc.tile_pool(name="xp", bufs=len(_PIECES)))
    sqp = ctx.enter_context(tc.tile_pool(name="sqp", bufs=3))
    rows = ctx.enter_context(tc.tile_pool(name="rows", bufs=4))
    sm = ctx.enter_context(tc.tile_pool(name="sm", bufs=2))
    psa = ctx.enter_context(tc.tile_pool(name="psa", bufs=2 * SPC, space="PSUM"))
    pss = ctx.enter_context(tc.tile_pool(name="pss", bufs=2, space="PSUM"))

    # ---- phase 0: input streams + constants ------------------------------
    xts = []
    for k, (s, c, o, ln) in enumerate(_PIECES):
        xt = xp.tile([128, ln], MM_DT, tag="x", name=f"x{k}")
        eng = nc.sync if k in _SYNC_KS else nc.scalar
        eng.dma_start(out=xt[:], in_=x[s, 128 * c : 128 * (c + 1), o : o + ln])
        xts.append(xt)

    # Constants ride SWDGE so the two HWDGE rings stay pure x-stream.
    band = consts.tile([128, 15], MM_DT)
    nc.gpsimd.dma_start(out=band[:], in_=hband.ap())
    band64 = consts.tile([64, 64], FP32)
    nc.gpsimd.dma_start(out=band64[:], in_=vband.ap())
    maskt = consts.tile([64, SPC, 64], FP32)
    nc.gpsimd.dma_start(out=maskt[:], in_=mask32.ap())

    ones64 = consts.tile([64, 64], FP32)
    nc.gpsimd.memset(ones64[:], 1.0)

    # Column-padded S tiles (cols 0 and 65 stay zero) and the shared rowsum.
    sbp = [consts.tile([64, 66], FP32, tag=f"sbp{s}", name=f"sbp{s}") for s in range(SPC)]
    for s in range(SPC):
        nc.gpsimd.memset(sbp[s][:, 0:1], 0.0)
        nc.gpsimd.memset(sbp[s][:, 65:66], 0.0)
    rowsum = consts.tile([64, SPC], FP32)

    ps_s = [psa.tile([8, 512], FP32, tag="acc", name=f"ps_s{i}") for i in range(SPC)]
    ps_q = [psa.tile([8, 512], FP32, tag="acc", name=f"ps_q{i}") for i in range(SPC)]

    spat = {}  # per-sample state carried from part 1 to part 2

    # ---- phase A: streamed channel reductions ----------------------------
    done = [0] * SPC  # per-sample pixels*chunks streamed so far (full = 2N)
    for k, (s, c, o, ln) in enumerate(_PIECES):
        xt = xts[k]
        first, last = done[s] == 0, done[s] + ln == 2 * N
        nblk = (ln + 511) // 512
        for b in range(nblk):
            f0, f1 = 512 * b, min(512 * b + 512, ln)
            j = (o + f0) // 512
            lo = o + f0 - 512 * j
            nc.tensor.matmul(
                ps_s[s][:, lo : lo + f1 - f0],
                band[:, 7 - j : 15 - j],
                xt[:, f0:f1],
                start=first and b == 0,
                stop=last and b == nblk - 1,
            )
        sq = sqp.tile([128, ln], MM_DT, tag="sq", name=f"sq{k}")
        xf = xt[:].bitcast(FP32)
        if k in _DVE_SQ_KS:
            nc.vector.tensor_mul(sq[:], xf, xf)
        else:
            nc.scalar.activation(sq[:], xf, AF.Square)
        for b in range(nblk):
            f0, f1 = 512 * b, min(512 * b + 512, ln)
            j = (o + f0) // 512
            lo = o + f0 - 512 * j
            nc.tensor.matmul(
                ps_q[s][:, lo : lo + f1 - f0],
                band[:, 7 - j : 15 - j],
                sq[:, f0:f1],
                start=first and b == 0,
                stop=last and b == nblk - 1,
            )
        done[s] += ln
        if done[s] == 2 * N:
            _spatial_a(tc, spat, s, ps_s[s], ps_q[s], band64, sbp[s], rows, sm, pss)
            _spatial_b(tc, spat, s, maskt, sm)
            if s == SPC - 1:
                _spatial_finish(tc, spat, s, ones64, rowsum, sm, pss, out)
        elif s == SPC - 1 and done[s] == N:
            # Mid-stream slot: earlier samples' exp/normalize/store, placed
            # here so the scalar stream reaches it after the dependencies
            # (their vector chains) have cleared.
            for t in range(SPC - 1):
                _spatial_finish(tc, spat, t, ones64, rowsum, sm, pss, out)


def _spatial_a(tc, spat, s, ps_s, ps_q, band64, sbp, rows, sm, pss):
    """Reshape + box filter for one sample, through BOX and T = box*s.

    The horizontal taps and cosine chain live in _spatial_b."""
    nc = tc.nc
    tail = s == SPC - 1

    # PSUM -> SBUF, then reshape [8,512] -> [64,64] (both APs enumerate the
    # 4096 pixels in order).  The s-reshape lands in the padded S tile.
    s_sb = rows.tile([8, 512], FP32, tag="srow", name=f"s_sb{s}")
    nc.vector.tensor_copy(s_sb[:], ps_s[:])
    q_sb = rows.tile([8, 512], FP32, tag="qrow", name=f"q_sb{s}")
    nc.scalar.copy(q_sb[:], ps_q[:])
    Qt = sm.tile([64, 64], FP32, tag="Qt", name=f"Qt{s}")
    if tail:
        nc.sync.dma_start(out=sbp[:, 1:65], in_=s_sb[:])
        nc.scalar.dma_start(out=Qt[:], in_=q_sb[:])
    else:
        nc.gpsimd.dma_start(out=sbp[:, 1:65], in_=s_sb[:])
        nc.gpsimd.dma_start(out=Qt[:], in_=q_sb[:])

    # Vertical 3-tap via tridiagonal stationary (pad columns stay zero
    # through the matmul), horizontal taps via free-shifted adds.
    v_ps = pss.tile([64, 66], FP32, tag="spat", name=f"v_ps{s}")
    nc.tensor.matmul(v_ps[:], band64[:], sbp[:], start=True, stop=True)
    Hb = sm.tile([64, 66], FP32, tag="Hb", name=f"Hb{s}")
    nc.scalar.copy(Hb[:], v_ps[:])
    spat[s] = (Hb, Qt, sbp)


def _spatial_b(tc, spat, s, maskt, sm):
    """Cosine chain for one sample: D, quake rsqrt, through exp input U2.

    Always on the vector engine; for non-tail samples this is emitted in a
    mid-stream slot so squares never queue behind it."""
    nc = tc.nc
    Hb, Qt, sbp = spat[s]

    # Horizontal 3-tap via free-shifted adds of the vertical-sum tile.
    T1 = sm.tile([64, 64], FP32, tag="T1", name=f"T1{s}")
    nc.vector.tensor_add(T1[:], Hb[:, 0:64], Hb[:, 1:65])
    BOX = sm.tile([64, 64], FP32, tag="BOX", name=f"BOX{s}")
    nc.vector.tensor_add(BOX[:], T1[:], Hb[:, 2:66])
    # T = box * s off the critical chain on gpsimd.
    T = sm.tile([64, 64], FP32, tag="T", name=f"T{s}")
    nc.gpsimd.tensor_mul(T[:], BOX[:], sbp[:, 1:65])

    # D = max(box^2 * q * 256/81, eps^2)
    P = sm.tile([64, 64], FP32, tag="P", name=f"P{s}")
    nc.vector.scalar_tensor_tensor(
        P[:], BOX[:], 256.0 / 81.0, BOX[:], op0=ALU.mult, op1=ALU.mult
    )
    P2 = sm.tile([64, 64], FP32, tag="P2", name=f"P2{s}")
    nc.vector.tensor_mul(P2[:], P[:], Qt[:])
    Dt = sm.tile([64, 64], FP32, tag="Dt", name=f"Dt{s}")
    nc.vector.tensor_scalar_max(Dt[:], P2[:], EPS2)

    # rsqrt(D): quake int-shift seed + one fused Newton step.
    ti = sm.tile([64, 64], I32, tag="ti", name=f"ti{s}")
    nc.vector.tensor_scalar(
        ti[:], Dt[:].bitcast(I32), 1, None, op0=ALU.logical_shift_right
    )
    yi = sm.tile([64, 64], I32, tag="yi", name=f"yi{s}")
    nc.vector.tensor_scalar(
        yi[:], ti[:], -1, QUAKE_2MAGIC // 2, op0=ALU.mult, op1=ALU.add
    )
    y0 = yi[:].bitcast(FP32)
    if NR_STEPS:
        a = sm.tile([64, 64], FP32, tag="nra", name=f"nra{s}")
        nc.vector.tensor_mul(a[:], y0, y0)
        w = sm.tile([64, 64], FP32, tag="nrw", name=f"nrw{s}")
        nc.vector.scalar_tensor_tensor(
            w[:], a[:], -0.5, Dt[:], op0=ALU.mult, op1=ALU.mult
        )
        y1 = sm.tile([64, 64], FP32, tag="nry", name=f"nry{s}")
        nc.vector.scalar_tensor_tensor(
            y1[:], w[:], 1.5, y0, op0=ALU.add, op1=ALU.mult
        )
        y0 = y1[:]

    # U2 = box*s*rsqrt + 1e30*mask  (exp(-U2/9) = masked exp(-sim))
    U = sm.tile([64, 64], FP32, tag="U", name=f"U{s}")
    nc.vector.tensor_mul(U[:], T[:], y0)
    U2 = sm.tile([64, 64], FP32, tag="U2", name=f"U2{s}")
    nc.vector.tensor_add(U2[:], U[:], maskt[:, s, :])
    spat[s] = U2


def _spatial_finish(tc, spat, s, ones64, rowsum, sm, pss, out):
    """exp, per-sample total, normalize, store."""
    nc = tc.nc
    U2 = spat[s]
    EM = sm.tile([64, 64], FP32, tag="EM", name=f"EM{s}")
    nc.scalar.activation(
        EM[:], U2[:], AF.Exp, scale=-1.0 / 9.0, accum_out=rowsum[:, s : s + 1]
    )
    # Total broadcast to all 64 partitions in one ones-matmul.
    totb = pss.tile([64, 1], FP32, tag="spat", name=f"totb{s}")
    nc.tensor.matmul(totb[:], ones64[:], rowsum[:, s : s + 1], start=True, stop=True)
    rec = sm.tile([64, 1], FP32, tag="rec", name=f"rec{s}")
    nc.vector.reciprocal(rec[:], totb[:])
    OUTt = sm.tile([64, 64], FP32, tag="OUTt", name=f"OUTt{s}")
    nc.vector.tensor_scalar_mul(OUTt[:], EM[:], rec[:, 0:1])
    eng = nc.scalar if s == SPC - 1 else nc.gpsimd
    eng.dma_start(out=out[:, s, :], in_=OUTt[:])


_NC_CACHE = {}


def _build():
    key = f"v5-nr{NR_STEPS}"
    if key in _NC_CACHE:
        return _NC_CACHE[key]
    nc = bacc.Bacc("TRN2", target_bir_lowering=False, debug=False)
    x = nc.declare_dram_parameter("x", [SPC, C, N], MM_DT, isOutput=False)
    mask32 = nc.declare_dram_parameter("mask32", [64, SPC, 64], FP32, isOutput=False)
    vband = nc.declare_dram_parameter("vband", [64, 64], FP32, isOutput=False)
    hband = nc.declare_dram_parameter("hband", [128, 15], MM_DT, isOutput=False)
    out = nc.declare_dram_parameter("out", [64, SPC, 64], FP32, isOutput=True)
    from contextlib import ExitStack

    with tile.TileContext(nc) as tc, ExitStack() as ctx:
        _kernel_body(ctx, tc, x, mask32, vband, hband, out)
    nc.compile()
    _NC_CACHE[key] = nc
    return nc


def band_matrix() -> np.ndarray:
    idx = np.arange(64)
    return (np.abs(idx[:, None] - idx[None, :]) <= 1).astype(np.float32)


def ind_band() -> np.ndarray:
    b = np.zeros((128, 15), dtype=np.float32)
    b[:, 7] = 1.0
    return b


def make_in_maps(x: np.ndarray, prev_drop_mask: np.ndarray) -> list:
    xs = np.ascontiguousarray(np.asarray(x), dtype=np.float32).reshape(B, C, N)
    # [B, N] bool -> per-core [64 rows, SPC, 64 cols] f32 pre-scaled +1e30.
    m32 = (np.asarray(prev_drop_mask).astype(np.float32) * 1e30).reshape(B, H, W)
    vb = band_matrix()
    hb = ind_band()
    maps = []
    for i in range(NCORES):
        mc = np.ascontiguousarray(m32[i * SPC : (i + 1) * SPC].transpose(1, 0, 2))
        maps.append(
            {
                "x": xs[i * SPC : (i + 1) * SPC],
                "mask32": mc,
                "vband": vb,
                "hband": hb,
            }
        )
    return maps


def gather_out(results) -> np.ndarray:
    # Each core returns [64, SPC, 64]; reorder to [B, H, W].
    outs = [
        np.asarray(results[i]["out"]).transpose(1, 0, 2) for i in range(NCORES)
    ]
    return np.concatenate(outs, axis=0).reshape(B, H, W)


def kernel(x: np.ndarray, prev_drop_mask: np.ndarray) -> np.ndarray:
    nc = _build()
    res = run_bass_kernel_spmd(nc, make_in_maps(x, prev_drop_mask), list(range(NCORES)))
    return gather_out(res.results)


# revision 24
# speedup vs baseline: 1.2392x; 1.0145x over previous
"""Trainium2 Bass kernel for LocalSpatialSimilarity.

Per sample (B=16, C=256, H=W=64, N=4096 pixels):
  s[p]  = sum_c x[c,p]                (channel sum)
  q[p]  = sum_c x[c,p]^2              (channel sum of squares)
  box   = 3x3 zero-padded box-sum of s (reshaped to 64x64)
  D     = max(box^2 * q * 256/81, 1e-12)
  sim   = (box * s / 9) * rsqrt(D)
  out   = softmax over p of (mask ? -inf : -sim)
        = (mask ? 0 : exp(-sim)) / total        (sim bounded in [-1,1] -> no
                                                 max-subtraction needed)

Sharding: pure data parallel, 2 samples per core across 8 cores.

Pipeline: x streams in as eleven [128, <=2048] float32r pieces, all on the
sync HWDGE ring with nothing ahead of them (a single clean ring measures
~350 GB/s; splitting across both rings thrashed HBM down to ~210).  Per
piece: channel-sum matmuls on the tensor engine straight off the raw piece
(float32r, one PE pass), squares on scalar/vector engines, then
sum-of-squares matmuls.  The band stationary trick lands 512-pixel blocks on
psum partitions so each sample accumulates into one [8, 512] psum tile per
quantity.  The last pieces shrink to 256 pixels so the post-stream tail is
short.

The spatial phase runs per sample.  Sample 0's phase hides under sample 1's
DMA stream; its small DMAs ride SWDGE (gpsimd) so the compute rings stay
clean, and its exp/normalize tail is emitted in a mid-stream slot so the
in-order scalar/vector engines never stall on it.  rsqrt is a Quake-style
int-shift seed plus one scalar_tensor_tensor-fused Newton step on the vector
engine, which keeps every scalar-engine function inside the exp_and_others
activation-table set: exactly one table load, none on the critical path.
"""

import sys

sys.path.insert(0, "/opt/trn_rl_repo")

import numpy as np

import concourse.bacc as bacc
import concourse.mybir as mybir
import concourse.tile as tile
from concourse.bass_utils import run_bass_kernel_spmd

B, C, H, W = 16, 256, 64, 64
N = H * W
NCORES = 8
SPC = B // NCORES  # samples per core
EPS2 = 1e-12
FP32 = mybir.dt.float32
I32 = mybir.dt.int32

# float32r: relaxed-precision fp32 matmul, single PE pass (plain fp32 = two).
MM_DT = mybir.dt.float32r
# Quake rsqrt seed: y = bitcast((0xBE6EB3BE - bitcast(D)) >> 1), one NR step.
QUAKE_2MAGIC = 0xBE6EB3BE  # 2 * 0x5F3759DF
NR_STEPS = 1  # one Newton step: 4.4e-4 output err (seed-only: 8.5e-3)

AF = mybir.ActivationFunctionType
ALU = mybir.AluOpType

# x pieces (sample, channel-chunk, pixel offset, length) in stream order.
# Sample SPC-1's chunk-1 pieces shrink toward the end: the last piece gates
# the kernel tail.
_PIECES = []
for _s in range(SPC):
    for _c in range(2):
        if _s == SPC - 1 and _c == 1:
            spans = [(0, 2048), (2048, 1024), (3072, 512), (3584, 256), (3840, 256)]
        else:
            spans = [(0, 2048), (2048, 2048)]
        for _o, _l in spans:
            _PIECES.append((_s, _c, _o, _l))
_LAST = len(_PIECES) - 1

# All x pieces ride the sync ring: splitting the stream across both HWDGE
# rings was measured MUCH slower (two packet-interleaved streams thrash HBM:
# 211 GB/s combined vs 294 single-ring).  The scalar ring stays empty for
# the tail sample's low-latency reshape/store smalls.
_SYNC_KS = set(range(len(_PIECES)))
# Square engines: vector for sample-0's odd pieces and the last piece
# (the vector engine runs the spatial chains, so sample-1 squares must not
# queue behind them); gpsimd takes two mid-stream sample-1 squares to keep
# the scalar engine's square backlog inside the piece-arrival window.
_DVE_SQ_KS = {1, 3, _LAST}
_POOL_SQ_KS = {5, 7}


def _kernel_body(ctx, tc, x, mask32, vband, hband, out):
    nc = tc.nc

    consts = ctx.enter_context(tc.tile_pool(name="consts", bufs=1))
    xp = ctx.enter_context(tc.tile_pool(name="xp", bufs=len(_PIECES)))
    sqp = ctx.enter_context(tc.tile_pool(name="sqp", bufs=3))
    rows = ctx.enter_context(tc.tile_pool(name="rows", bufs=4))
    sm = ctx.enter_context(tc.tile_pool(name="sm", bufs=2))
    psa = ctx.enter_context(tc.tile_pool(name="psa", bufs=2 * SPC, space="PSUM"))
    pss = ctx.enter_context(tc.tile_pool(name="pss", bufs=2, space="PSUM"))

    # ---- phase 0: input streams + constants ------------------------------
    xts = []
    for k, (s, c, o, ln) in enumerate(_PIECES):
        xt = xp.tile([128, ln], MM_DT, tag="x", name=f"x{k}")
        eng = nc.sync if k in _SYNC_KS else nc.scalar
        eng.dma_start(out=xt[:], in_=x[s, 128 * c : 128 * (c + 1), o : o + ln])
        xts.append(xt)

    # Constants ride SWDGE so the two HWDGE rings stay pure x-stream.
    band = consts.tile([128, 15], MM_DT)
    nc.gpsimd.dma_start(out=band[:], in_=hband.ap())
    band64 = consts.tile([64, 64], FP32)
    nc.gpsimd.dma_start(out=band64[:], in_=vband.ap())
    maskt = consts.tile([64, SPC, 64], FP32)
    nc.gpsimd.dma_start(out=maskt[:], in_=mask32.ap())

    ones64 = consts.tile([64, 64], FP32)
    nc.gpsimd.memset(ones64[:], 1.0)

    # Column-padded S tiles (cols 0 and 65 stay zero) and the shared rowsum.
    sbp = [consts.tile([64, 66], FP32, tag=f"sbp{s}", name=f"sbp{s}") for s in range(SPC)]
    for s in range(SPC):
        nc.gpsimd.memset(sbp[s][:, 0:1], 0.0)
        nc.gpsimd.memset(sbp[s][:, 65:66], 0.0)
    rowsum = consts.tile([64, SPC], FP32)

    ps_s = [psa.tile([8, 512], FP32, tag="acc", name=f"ps_s{i}") for i in range(SPC)]
    ps_q = [psa.tile([8, 512], FP32, tag="acc", name=f"ps_q{i}") for i in range(SPC)]

    spat = {}  # per-sample state carried from part 1 to part 2

    # ---- phase A: streamed channel reductions ----------------------------
    done = [0] * SPC  # per-sample pixels*chunks streamed so far (full = 2N)
    for k, (s, c, o, ln) in enumerate(_PIECES):
        xt = xts[k]
        first, last = done[s] == 0, done[s] + ln == 2 * N
        nblk = (ln + 511) // 512
        for b in range(nblk):
            f0, f1 = 512 * b, min(512 * b + 512, ln)
            j = (o + f0) // 512
            lo = o + f0 - 512 * j
            nc.tensor.matmul(
                ps_s[s][:, lo : lo + f1 - f0],
                band[:, 7 - j : 15 - j],
                xt[:, f0:f1],
                start=first and b == 0,
                stop=last and b == nblk - 1,
            )
        sq = sqp.tile([128, ln], MM_DT, tag="sq", name=f"sq{k}")
        xf = xt[:].bitcast(FP32)
        if k in _DVE_SQ_KS:
            nc.vector.tensor_mul(sq[:], xf, xf)
        elif k in _POOL_SQ_KS:
            nc.gpsimd.tensor_mul(sq[:], xf, xf)
        else:
            nc.scalar.activation(sq[:], xf, AF.Square)
        for b in range(nblk):
            f0, f1 = 512 * b, min(512 * b + 512, ln)
            j = (o + f0) // 512
            lo = o + f0 - 512 * j
            nc.tensor.matmul(
                ps_q[s][:, lo : lo + f1 - f0],
                band[:, 7 - j : 15 - j],
                sq[:, f0:f1],
                start=first and b == 0,
                stop=last and b == nblk - 1,
            )
        done[s] += ln
        if done[s] == 2 * N:
            _spatial_a(tc, spat, s, ps_s[s], ps_q[s], band64, sbp[s], rows, sm, pss)
            _spatial_b(tc, spat, s, maskt, sm)
            if s == SPC - 1:
                _spatial_finish(tc, spat, s, ones64, rowsum, sm, pss, out)
        elif s == SPC - 1 and done[s] == N:
            # Mid-stream slot: earlier samples' exp/normalize/store, placed
            # here so the scalar stream reaches it after the dependencies
            # (their vector chains) have cleared.
            for t in range(SPC - 1):
                _spatial_finish(tc, spat, t, ones64, rowsum, sm, pss, out)


def _spatial_a(tc, spat, s, ps_s, ps_q, band64, sbp, rows, sm, pss):
    """Reshape + box filter for one sample, through BOX and T = box*s.

    The horizontal taps and cosine chain live in _spatial_b."""
    nc = tc.nc
    tail = s == SPC - 1

    # PSUM -> SBUF, then reshape [8,512] -> [64,64] (both APs enumerate the
    # 4096 pixels in order).  The s-reshape lands in the padded S tile.
    s_sb = rows.tile([8, 512], FP32, tag="srow", name=f"s_sb{s}")
    nc.vector.tensor_copy(s_sb[:], ps_s[:])
    q_sb = rows.tile([8, 512], FP32, tag="qrow", name=f"q_sb{s}")
    nc.scalar.copy(q_sb[:], ps_q[:])
    Qt = sm.tile([64, 64], FP32, tag="Qt", name=f"Qt{s}")
    if tail:
        nc.sync.dma_start(out=sbp[:, 1:65], in_=s_sb[:])
        nc.scalar.dma_start(out=Qt[:], in_=q_sb[:])
    else:
        nc.gpsimd.dma_start(out=sbp[:, 1:65], in_=s_sb[:])
        nc.gpsimd.dma_start(out=Qt[:], in_=q_sb[:])

    # Vertical 3-tap via tridiagonal stationary (pad columns stay zero
    # through the matmul), horizontal taps via free-shifted adds.
    v_ps = pss.tile([64, 66], FP32, tag="spat", name=f"v_ps{s}")
    nc.tensor.matmul(v_ps[:], band64[:], sbp[:], start=True, stop=True)
    Hb = sm.tile([64, 66], FP32, tag="Hb", name=f"Hb{s}")
    nc.scalar.copy(Hb[:], v_ps[:])
    spat[s] = (Hb, Qt, sbp)


def _spatial_b(tc, spat, s, maskt, sm):
    """Cosine chain for one sample: D, quake rsqrt, through exp input U2.

    Always on the vector engine; for non-tail samples this is emitted in a
    mid-stream slot so squares never queue behind it."""
    nc = tc.nc
    Hb, Qt, sbp = spat[s]

    # Horizontal 3-tap via free-shifted adds of the vertical-sum tile.
    T1 = sm.tile([64, 64], FP32, tag="T1", name=f"T1{s}")
    nc.vector.tensor_add(T1[:], Hb[:, 0:64], Hb[:, 1:65])
    BOX = sm.tile([64, 64], FP32, tag="BOX", name=f"BOX{s}")
    nc.vector.tensor_add(BOX[:], T1[:], Hb[:, 2:66])
    # T = box * s: gpsimd for the tail sample (idle then); vector for
    # sample 0 so the Pool square stream never queues behind it.
    T = sm.tile([64, 64], FP32, tag="T", name=f"T{s}")
    teng = nc.gpsimd if s == SPC - 1 else nc.vector
    teng.tensor_mul(T[:], BOX[:], sbp[:, 1:65])

    # D = max(box^2 * q * 256/81, eps^2)
    P = sm.tile([64, 64], FP32, tag="P", name=f"P{s}")
    nc.vector.scalar_tensor_tensor(
        P[:], BOX[:], 256.0 / 81.0, BOX[:], op0=ALU.mult, op1=ALU.mult
    )
    P2 = sm.tile([64, 64], FP32, tag="P2", name=f"P2{s}")
    nc.vector.tensor_mul(P2[:], P[:], Qt[:])
    Dt = sm.tile([64, 64], FP32, tag="Dt", name=f"Dt{s}")
    nc.vector.tensor_scalar_max(Dt[:], P2[:], EPS2)

    # rsqrt(D): quake int-shift seed + one fused Newton step.
    ti = sm.tile([64, 64], I32, tag="ti", name=f"ti{s}")
    nc.vector.tensor_scalar(
        ti[:], Dt[:].bitcast(I32), 1, None, op0=ALU.logical_shift_right
    )
    yi = sm.tile([64, 64], I32, tag="yi", name=f"yi{s}")
    nc.vector.tensor_scalar(
        yi[:], ti[:], -1, QUAKE_2MAGIC // 2, op0=ALU.mult, op1=ALU.add
    )
    y0 = yi[:].bitcast(FP32)
    if NR_STEPS:
        a = sm.tile([64, 64], FP32, tag="nra", name=f"nra{s}")
        nc.vector.tensor_mul(a[:], y0, y0)
        w = sm.tile([64, 64], FP32, tag="nrw", name=f"nrw{s}")
        nc.vector.scalar_tensor_tensor(
            w[:], a[:], -0.5, Dt[:], op0=ALU.mult, op1=ALU.mult
        )
        y1 = sm.tile([64, 64], FP32, tag="nry", name=f"nry{s}")
        nc.vector.scalar_tensor_tensor(
            y1[:], w[:], 1.5, y0, op0=ALU.add, op1=ALU.mult
        )
        y0 = y1[:]

    # U2 = box*s*rsqrt + 1e30*mask  (exp(-U2/9) = masked exp(-sim))
    U = sm.tile([64, 64], FP32, tag="U", name=f"U{s}")
    nc.vector.tensor_mul(U[:], T[:], y0)
    U2 = sm.tile([64, 64], FP32, tag="U2", name=f"U2{s}")
    nc.vector.tensor_add(U2[:], U[:], maskt[:, s, :])
    spat[s] = U2


def _spatial_finish(tc, spat, s, ones64, rowsum, sm, pss, out):
    """exp, per-sample total, normalize, store."""
    nc = tc.nc
    U2 = spat[s]
    EM = sm.tile([64, 64], FP32, tag="EM", name=f"EM{s}")
    nc.scalar.activation(
        EM[:], U2[:], AF.Exp, scale=-1.0 / 9.0, accum_out=rowsum[:, s : s + 1]
    )
    # Total broadcast to all 64 partitions in one ones-matmul.
    totb = pss.tile([64, 1], FP32, tag="spat", name=f"totb{s}")
    nc.tensor.matmul(totb[:], ones64[:], rowsum[:, s : s + 1], start=True, stop=True)
    rec = sm.tile([64, 1], FP32, tag="rec", name=f"rec{s}")
    nc.vector.reciprocal(rec[:], totb[:])
    OUTt = sm.tile([64, 64], FP32, tag="OUTt", name=f"OUTt{s}")
    nc.vector.tensor_scalar_mul(OUTt[:], EM[:], rec[:, 0:1])
    nc.scalar.dma_start(out=out[:, s, :], in_=OUTt[:])


_NC_CACHE = {}


def _build():
    key = f"v6-nr{NR_STEPS}"
    if key in _NC_CACHE:
        return _NC_CACHE[key]
    nc = bacc.Bacc("TRN2", target_bir_lowering=False, debug=False)
    x = nc.declare_dram_parameter("x", [SPC, C, N], MM_DT, isOutput=False)
    mask32 = nc.declare_dram_parameter("mask32", [64, SPC, 64], FP32, isOutput=False)
    vband = nc.declare_dram_parameter("vband", [64, 64], FP32, isOutput=False)
    hband = nc.declare_dram_parameter("hband", [128, 15], MM_DT, isOutput=False)
    out = nc.declare_dram_parameter("out", [64, SPC, 64], FP32, isOutput=True)
    from contextlib import ExitStack

    with tile.TileContext(nc) as tc, ExitStack() as ctx:
        _kernel_body(ctx, tc, x, mask32, vband, hband, out)
    nc.compile()
    _NC_CACHE[key] = nc
    return nc


def band_matrix() -> np.ndarray:
    idx = np.arange(64)
    return (np.abs(idx[:, None] - idx[None, :]) <= 1).astype(np.float32)


def ind_band() -> np.ndarray:
    b = np.zeros((128, 15), dtype=np.float32)
    b[:, 7] = 1.0
    return b


def make_in_maps(x: np.ndarray, prev_drop_mask: np.ndarray) -> list:
    xs = np.ascontiguousarray(np.asarray(x), dtype=np.float32).reshape(B, C, N)
    # [B, N] bool -> per-core [64 rows, SPC, 64 cols] f32 pre-scaled +1e30.
    m32 = (np.asarray(prev_drop_mask).astype(np.float32) * 1e30).reshape(B, H, W)
    vb = band_matrix()
    hb = ind_band()
    maps = []
    for i in range(NCORES):
        mc = np.ascontiguousarray(m32[i * SPC : (i + 1) * SPC].transpose(1, 0, 2))
        maps.append(
            {
                "x": xs[i * SPC : (i + 1) * SPC],
                "mask32": mc,
                "vband": vb,
                "hband": hb,
            }
        )
    return maps


def gather_out(results) -> np.ndarray:
    # Each core returns [64, SPC, 64]; reorder to [B, H, W].
    outs = [
        np.asarray(results[i]["out"]).transpose(1, 0, 2) for i in range(NCORES)
    ]
    return np.concatenate(outs, axis=0).reshape(B, H, W)


def kernel(x: np.ndarray, prev_drop_mask: np.ndarray) -> np.ndarray:
    nc = _build()
    res = run_bass_kernel_spmd(nc, make_in_maps(x, prev_drop_mask), list(range(NCORES)))
    return gather_out(res.results)
